# revision 15
# baseline (speedup 1.0000x reference)
"""Trainium2 Bass kernel for nn_MultiHeadAttention_8684423872640.

Math: the reference collapses algebraically. With
  s[m]   = Wfc[0, m // 64] / sqrt(64)
  Abar   = (Wk * s[:,None]).T @ Wq / L          # [1024, 1024] weights-only
  u      = Wk.T @ (s * bq)                      # [1024]
  qv     = Wq.T @ (s * bk) / L                  # [1024]
  c0     = (s * bk) @ bq + bfc[0]
the output for batch b is
  xsum_b = sum_l x[b, l, :]                     # [1024]
  w_eff  = Abar @ xsum_b + u                    # [1024]
  c      = qv @ xsum_b + c0
  out[b, l, 0] = x[b, l, :] @ w_eff + c

Sharding: data-parallel over B -- core c handles batch c.

v2 pipeline (per core):
  - x ships fp8-e4m3 [N=1024, L=4096] as 4 pair tiles [128, 2L]; rings:
    sync x0,x2,x4,x6,x7b / scalar x1,x3,x5,x7a / gps at0,qv,u,c0,at1.
    All DMA issues pinned to queue fronts (descgen done before data
    arrives, so compute engines' queues are clean in steady state).
  - Row sums: per tile, three zones reduced in parallel, one op each:
    ACT activation(Copy, accum_out) on raw fp8; DVE and GPS each do
    scalar_tensor_tensor(halves, op=add, accum_out) -- pairwise add +
    free-axis accumulate in a single pass (2 cols/cycle consumed).
    GPS combines the three fp32 partials into fp16 xm in one stt op.
  - Folds: per tile pt, 8 closed-group MMs (at fp8 x xm fp16) into a
    single PSUM bank (cols pt*8+nt) + qv MM chained into cps.  One DVE
    tensor_reduce at the end sums all 64 fold columns -> w8acc, then
    one stt adds u and rescales into the pass-2 w dtype.
  - PE warmup MMs gated on x0/x2 keep the HAM clock ramping through
    the reduce window (fillers per fold as in v1).
  - c broadcast via PE ones-matmul + ACT copy.
  - Pass-2 (KERNEL_P2=dr, default): DoubleRow fp8xfp8 matvec -- 32 MMs
    of [128,2,512] pairs at tile_position (0,32j), 2 PSUM waves; w in
    fp8 scaled 2^18.  KERNEL_P2=mx falls back to v1's 64 mixed-dtype
    MMs (bf16 w, scale 2^20).
  - Epilogue per wave (tensor_scalar mult+add c) overlaps wave 2 MMs;
    two simple 4-descriptor out-DMAs on the sync ring.
"""

import os
import sys
import functools
import numpy as np

B, L, N = 8, 4096, 1024
D_K = 64
NCORES = 8
PT = N // 128   # 8 feature tiles
LCH = 512       # pass-2 moving chunk (PSUM bank limit)
H = L // 2

_TRN_REPO = "/opt/trn_rl_repo"


def _ensure_path():
    if _TRN_REPO not in sys.path and os.path.isdir(_TRN_REPO):
        sys.path.insert(0, _TRN_REPO)


# pass-2 is bf16-w-stationary x fp8-moving, 4-way column-tiled: 4 fp8
# moving cols/cycle, the PE moving-bus limit.  (DoubleRow fp8xfp8 was
# tried and is REJECTED by walrus for tile_position j>0
# (s3d3_mm_valid_dst_partition); untiled DR is only 2 cols/cycle, so it
# cannot beat 4-way column tiling.)
# reduce zone widths per full tile (ACT raw / DVE stt / GPS stt)
# reduce zones per tile: ACT raw-accum [0:ZA), DVE stt pair-add+accum
# [ZA:4096). GPSIMD cannot run TensorScalarPtr (walrus opcode-on-engine
# check), and ACT+DVE alone already outpace the DMA x-rate.
_ZA = int(os.environ.get("KERNEL_ZA", "1424"))
_WARM1 = int(os.environ.get("KERNEL_WARM1", "6"))
_FILL = int(os.environ.get("KERNEL_FILL", "2"))

WSCALE = float(2 ** 20)   # host scale on Abar/qv/u


@functools.lru_cache(maxsize=4)
def _build(za: int = _ZA, warm1: int = _WARM1, nfill: int = _FILL):
    _ensure_path()
    import concourse.bass as bass
    import concourse.tile as tile
    from concourse import bacc, mybir

    f32 = mybir.dt.float32
    bf16 = mybir.dt.bfloat16
    f16 = mybir.dt.float16
    f8 = mybir.dt.float8e4
    ADD = mybir.AluOpType.add
    MUL = mybir.AluOpType.mult
    COPY = mybir.ActivationFunctionType.Copy

    zd = L - za               # DVE stt zone
    zd2 = zd // 2
    # half-tile zones for tile 7 (2048 cols each half)
    ha = za // 2
    hd = H - ha

    nc = bacc.Bacc(
        "TRN2",
        target_bir_lowering=False,
        debug=False,
        enable_asserts=False,
        num_devices=NCORES,
    )

    xT = nc.dram_tensor("xT", [N, L], f8, kind="ExternalInput").ap()
    atr = nc.dram_tensor("atr", [128, PT * N], f8, kind="ExternalInput").ap()
    qv8 = nc.dram_tensor("qv8", [128, PT], f8, kind="ExternalInput").ap()
    u8 = nc.dram_tensor("u8", [128, PT], f32, kind="ExternalInput").ap()
    c0 = nc.dram_tensor("c0", [1, 1], f32, kind="ExternalInput").ap()
    out_d = nc.dram_tensor("out", [1, L], f32, kind="ExternalOutput").ap()

    with tile.TileContext(nc) as tc:
        with (
            tc.tile_pool(name="sb", bufs=1) as sb,
            tc.tile_pool(name="ps", bufs=1, space="PSUM") as ps,
        ):
            xp = [sb.tile([128, 2 * L], f8, tag=f"xp{k}", name=f"xp{k}")
                  for k in range(4)]
            # tile t lives at xp[t//2][:, (t%2)*L : (t%2+1)*L]
            xv = [xp[t // 2][:, (t % 2) * L:(t % 2 + 1) * L] for t in range(8)]
            at0_sb = sb.tile([128, 2 * N], f8, tag="at0")
            at1a_sb = sb.tile([128, 3 * N], f8, tag="at1a")
            at1b_sb = sb.tile([128, 3 * N], f8, tag="at1b")
            qv_sb = sb.tile([128, PT], f8, tag="qv")
            u_sb = sb.tile([128, PT], f32, tag="u")
            c0_sb = sb.tile([1, 1], f32, tag="c0")
            ones32 = sb.tile([1, 128], f32, tag="ones")
            scrA = sb.tile([128, za], f8, tag="scrA")
            scrD = sb.tile([128, zd2], f16, tag="scrD")
            parts = sb.tile([128, 8, 2], f32, tag="parts")
            parts7 = sb.tile([128, 4], f32, tag="parts7")
            xm7f = sb.tile([128, 1], f32, tag="xm7f")
            xm_all = sb.tile([128, PT], f16, tag="xm")
            w8acc = sb.tile([128, PT], f32, tag="w8acc")
            w_sb = sb.tile([128, PT], bf16, tag="weff", name="w_sb")
            c_sb = sb.tile([1, 1], f32, tag="csb")
            c_bc = sb.tile([128, 1], f32, tag="cbc")
            out_sb = sb.tile([128, 2 * LCH], f32, tag="osb")

            # PSUM: one tile per bank (pad free dim to a full 2KB bank)
            wp_all = ps.tile([128, 512], f32, tag="wp")     # cols 0:64 used
            warm = ps.tile([1, 512], f32, tag="warm")
            c_ps = ps.tile([1, 512], f32, tag="cps")        # [0:1,0:1] used
            cb_ps = ps.tile([128, 512], f32, tag="cbp")     # col 0 used
            o_ps = [ps.tile([128, LCH], f32, tag=f"o{w}", name=f"o{w}")
                    for w in range(2)]

            # ---- DMA: issues pinned to queue fronts ----
            # at rides mid-stream on both x rings: late x tiles then arrive
            # at a cadence the ACT+DVE reduce can absorb without backlog,
            # and x7b stays the last arrival (shortest possible tail).
            with tc.high_priority():
                nc.gpsimd.dma_start(at0_sb[:], atr[:, 0:2 * N])
                nc.gpsimd.dma_start(qv_sb[:], qv8[:])
                nc.gpsimd.dma_start(u_sb[:], u8[:])
                nc.gpsimd.dma_start(c0_sb[:], c0[:])
                for k in range(3):
                    nc.sync.dma_start(
                        xp[k][:, 0:L], xT[256 * k:256 * k + 128, :])
                nc.sync.dma_start(at1a_sb[:], atr[:, 2 * N:5 * N])
                nc.sync.dma_start(xp[3][:, 0:L], xT[768:896, :])
                nc.sync.dma_start(xp[3][:, L + H:2 * L], xT[896:, H:L])
                for k in range(3):
                    nc.scalar.dma_start(
                        xp[k][:, L:2 * L], xT[256 * k + 128:256 * (k + 1), :])
                nc.scalar.dma_start(at1b_sb[:], atr[:, 5 * N:])
                nc.scalar.dma_start(xp[3][:, L:L + H], xT[896:, 0:H])

            nc.gpsimd.memset(ones32[:], 1.0)
            # hoist ACT table load to t~0 via a dummy activation
            nc.scalar.activation(scrA[0:1, 0:8], ones32[0:1, 0:8], COPY,
                                 bias=0.0)
            # pre-zero pass-2 PSUM rows the matvec leaves unwritten (the
            # epilogue reads all 128 partitions; only rows 32j get data)
            for w in range(2):
                nc.vector.memset(o_ps[w][:, :], 0.0)

            # ---- per-tile row-sum zones (one op per engine) ----
            def reduce_tile(t):
                x_ = xv[t]
                nc.scalar.activation(
                    scrA[:, 0:za], x_[:, 0:za], COPY, bias=0.0,
                    accum_out=parts[:, t, 0:1])
                nc.vector.scalar_tensor_tensor(
                    scrD[:, 0:zd2], x_[:, za:za + zd2], 1.0,
                    x_[:, za + zd2:L], MUL, ADD,
                    accum_out=parts[:, t, 1:2])

            def reduce_half7(h):  # tile 7 halves -> parts7[:, 2h:2h+2]
                x_ = xv[7][:, h * H:(h + 1) * H]
                nc.scalar.activation(
                    scrA[:, 0:ha], x_[:, 0:ha], COPY, bias=0.0,
                    accum_out=parts7[:, 2 * h + 0:2 * h + 1])
                hd2 = hd // 2
                nc.vector.scalar_tensor_tensor(
                    scrD[:, 0:hd2], x_[:, ha:ha + hd2], 1.0,
                    x_[:, ha + hd2:H], MUL, ADD,
                    accum_out=parts7[:, 2 * h + 1:2 * h + 2])

            def combine(t):  # xm[t] = A + D on GPS, fp16
                nc.gpsimd.tensor_add(
                    xm_all[:, t:t + 1], parts[:, t, 0:1], parts[:, t, 1:2])

            # ---- folds ----
            def fold(pt):
                if pt < 2:
                    a_sb, off = at0_sb, pt
                elif pt < 5:
                    a_sb, off = at1a_sb, pt - 2
                else:
                    a_sb, off = at1b_sb, pt - 5
                for nt in range(PT):
                    nc.tensor.matmul(
                        wp_all[:, pt * 8 + nt:pt * 8 + nt + 1],
                        a_sb[:, off * N + nt * 128:off * N + (nt + 1) * 128],
                        xm_all[:, pt:pt + 1], start=True, stop=True)
                nc.tensor.matmul(
                    c_ps[0:1, 0:1], qv_sb[:, pt:pt + 1], xm_all[:, pt:pt + 1],
                    start=(pt == 0), stop=(pt == PT - 1))
                # PE fillers gated on this xm: keep the HAM clock ramping
                if pt < 6:
                    for i in range(nfill):
                        nc.tensor.matmul(
                            warm[0:1, :], xm_all[:, pt:pt + 1],
                            xv[pt][:, i * LCH:(i + 1) * LCH],
                            start=(i == 0), stop=(i == nfill - 1))

            # warmup burst as soon as x0 lands
            for i in range(warm1):
                nc.tensor.matmul(
                    warm[0:1, :], xv[0][:, 0:1], xv[0][:, 0:LCH],
                    start=(i == 0), stop=(i == warm1 - 1))

            # tiles 0..6 (pair-buffered), tile 7 in halves
            for t in range(7):
                reduce_tile(t)
                if t >= 1:
                    combine(t - 1)
                    fold(t - 1)
            reduce_half7(0)
            # late warm burst: x6 is resident; keep the PE pstate streak
            # alive across the fold-6/fold-7 window into pass-2
            for i in range(6):
                nc.tensor.matmul(
                    warm[0:1, :], xv[6][:, 1:2], xv[6][:, i * LCH:(i + 1) * LCH],
                    start=(i == 0), stop=(i == 5))
            combine(6)
            fold(6)
            reduce_half7(1)
            for i in range(3):
                nc.tensor.matmul(
                    warm[0:1, :], xv[7][:, 1:2], xv[7][:, i * LCH:(i + 1) * LCH],
                    start=(i == 0), stop=(i == 2))
            # pre-sum folds 0-6 plus u while tile 7 is still reducing: the
            # post-fold-7 critical chain is then a single stt off PSUM
            nc.vector.tensor_reduce(
                w8acc[:], wp_all[:, 0:56].rearrange("p (a b) -> p b a", a=7),
                axis=mybir.AxisListType.X, op=mybir.AluOpType.add)
            nc.vector.tensor_add(w8acc[:], w8acc[:], u_sb[:])
            # tile-7 combine on DVE (GPS may lag; DVE is free now)
            with nc.allow_low_precision("xm7 fp16 accumulate, 4 partials"):
                nc.vector.tensor_reduce(
                    xm_all[:, 7:8], parts7[:], axis=mybir.AxisListType.X,
                    op=mybir.AluOpType.add)
            fold(7)

            # ---- finalize w / c ----
            nc.vector.scalar_tensor_tensor(
                w_sb[:], wp_all[:, 56:64], 1.0, w8acc[:], MUL, ADD)
            nc.vector.tensor_scalar(
                c_sb[:], c_ps[0:1, 0:1], 1.0 / WSCALE, c0_sb[0:1, 0:1],
                MUL, ADD)
            # broadcast c to all partitions via PE + ACT copy
            nc.tensor.matmul(cb_ps[:, 0:1], ones32[:], c_sb[:],
                             start=True, stop=True)
            nc.scalar.activation(c_bc[:], cb_ps[:, 0:1], COPY, bias=0.0)

            # ---- pass 2 ----
            oscale = 1.0 / WSCALE
            for wave in range(2):
                for nt in range(PT):
                    for j in range(4):
                        lc = wave * 4 + j
                        nc.tensor.matmul(
                            o_ps[wave][32 * j:32 * j + 1, :],
                            w_sb[:, nt:nt + 1],
                            xv[nt][:, lc * LCH:(lc + 1) * LCH],
                            start=(nt == 0), stop=(nt == PT - 1),
                            tile_position=(0, 32 * j))
                nc.vector.tensor_scalar(
                    out_sb[:, wave * LCH:(wave + 1) * LCH], o_ps[wave][:, :],
                    oscale, c_bc[:, 0:1], MUL, ADD)
                nc.sync.dma_start(
                    out_d[0:1, wave * 4 * LCH:(wave + 1) * 4 * LCH]
                    .rearrange("p (j k) -> p j k", j=4),
                    out_sb[0:97:32, wave * LCH:(wave + 1) * LCH])

    nc.compile()
    return nc


def _prep_host(inputs):
    """Fold weights on host (f64 accumulate) and lay out per-core arrays."""
    import ml_dtypes

    Wq = np.asarray(inputs["Wq"], np.float64)
    bq = np.asarray(inputs["bq"], np.float64)
    Wk = np.asarray(inputs["Wk"], np.float64)
    bk = np.asarray(inputs["bk"], np.float64)
    Wfc = np.asarray(inputs["Wfc"], np.float64)
    bfc = np.asarray(inputs["bfc"], np.float64)

    s = np.repeat(Wfc[0], D_K) / np.sqrt(D_K)
    A = (Wk * s[:, None]).T @ Wq / L          # [n, p] ; w_eff = A @ xsum + u
    u = Wk.T @ (s * bq)
    qv = Wq.T @ (s * bk) / L
    c0 = float((s * bk) @ bq + bfc[0])

    f8 = ml_dtypes.float8_e4m3

    at = np.ascontiguousarray(A.T) * WSCALE
    atr = np.ascontiguousarray(
        at.reshape(PT, 128, N).transpose(1, 0, 2).reshape(128, PT * N)
    ).astype(f8)
    qv8 = np.ascontiguousarray(
        (qv * WSCALE).reshape(PT, 128).T).astype(f8)
    u8 = np.ascontiguousarray(
        (u * WSCALE).reshape(PT, 128).T).astype(np.float32)
    c0a = np.full((1, 1), c0, np.float32)

    x = np.asarray(inputs["x"])
    shared = {"atr": atr, "qv8": qv8, "u8": u8, "c0": c0a}
    in_maps = []
    for c in range(NCORES):
        m = dict(shared)
        m["xT"] = np.ascontiguousarray(x[c].T).astype(f8)
        in_maps.append(m)
    return in_maps


LAST_RESULTS = None


def kernel(**inputs) -> np.ndarray:
    global LAST_RESULTS
    _ensure_path()
    from concourse.bass_utils import run_bass_kernel_spmd

    nc = _build()
    in_maps = _prep_host(inputs)
    kw = {}
    if os.environ.get("KERNEL_TRACE"):
        kw["trace"] = True
    res = run_bass_kernel_spmd(nc, in_maps, list(range(NCORES)), **kw)
    LAST_RESULTS = res
    out = np.stack([res.results[c]["out"].reshape(L, 1) for c in range(NCORES)])
    return out.astype(np.float32)


if __name__ == "__main__":
    rng = np.random.default_rng(0)
    demo = {
        "x": rng.standard_normal((B, L, N), np.float32),
        "Wq": rng.standard_normal((N, N), np.float32) * 0.03,
        "bq": rng.standard_normal((N,), np.float32) * 0.03,
        "Wk": rng.standard_normal((N, N), np.float32) * 0.03,
        "bk": rng.standard_normal((N,), np.float32) * 0.03,
        "Wfc": rng.standard_normal((1, 16), np.float32) * 0.25,
        "bfc": rng.standard_normal((1,), np.float32) * 0.25,
    }
    o = kernel(**demo)
    print("out", o.shape, o.dtype, float(np.abs(o).max()))


# revision 16
# speedup vs baseline: 1.1368x; 1.1368x over previous
"""Trainium2 Bass kernel for nn_MultiHeadAttention_8684423872640.

Math: the reference collapses algebraically. With
  s[m]   = Wfc[0, m // 64] / sqrt(64)
  Abar   = (Wk * s[:,None]).T @ Wq / L          # [1024, 1024] weights-only
  u      = Wk.T @ (s * bq)                      # [1024]
  qv     = Wq.T @ (s * bk) / L                  # [1024]
  c0     = (s * bk) @ bq + bfc[0]
the output for batch b is
  xsum_b = sum_l x[b, l, :]                     # [1024]
  w_eff  = Abar @ xsum_b + u                    # [1024]
  c      = qv @ xsum_b + c0
  out[b, l, 0] = x[b, l, :] @ w_eff + c

Sharding: data-parallel over B -- core c handles batch c.

v2 pipeline (per core):
  - x ships fp8-e4m3 [N=1024, L=4096] as 4 pair tiles [128, 2L]; rings:
    sync x0,x2,x4,x6,x7b / scalar x1,x3,x5,x7a / gps at0,qv,u,c0,at1.
    All DMA issues pinned to queue fronts (descgen done before data
    arrives, so compute engines' queues are clean in steady state).
  - Row sums: per tile, three zones reduced in parallel, one op each:
    ACT activation(Copy, accum_out) on raw fp8; DVE and GPS each do
    scalar_tensor_tensor(halves, op=add, accum_out) -- pairwise add +
    free-axis accumulate in a single pass (2 cols/cycle consumed).
    GPS combines the three fp32 partials into fp16 xm in one stt op.
  - Folds: per tile pt, 8 closed-group MMs (at fp8 x xm fp16) into a
    single PSUM bank (cols pt*8+nt) + qv MM chained into cps.  One DVE
    tensor_reduce at the end sums all 64 fold columns -> w8acc, then
    one stt adds u and rescales into the pass-2 w dtype.
  - PE warmup MMs gated on x0/x2 keep the HAM clock ramping through
    the reduce window (fillers per fold as in v1).
  - c broadcast via PE ones-matmul + ACT copy.
  - Pass-2 (KERNEL_P2=dr, default): DoubleRow fp8xfp8 matvec -- 32 MMs
    of [128,2,512] pairs at tile_position (0,32j), 2 PSUM waves; w in
    fp8 scaled 2^18.  KERNEL_P2=mx falls back to v1's 64 mixed-dtype
    MMs (bf16 w, scale 2^20).
  - Epilogue per wave (tensor_scalar mult+add c) overlaps wave 2 MMs;
    two simple 4-descriptor out-DMAs on the sync ring.
"""

import os
import sys
import functools
import numpy as np

B, L, N = 8, 4096, 1024
D_K = 64
NCORES = 8
PT = N // 128   # 8 feature tiles
LCH = 512       # pass-2 moving chunk (PSUM bank limit)
H = L // 2

_TRN_REPO = "/opt/trn_rl_repo"


def _ensure_path():
    if _TRN_REPO not in sys.path and os.path.isdir(_TRN_REPO):
        sys.path.insert(0, _TRN_REPO)


# pass-2 is bf16-w-stationary x fp8-moving, 4-way column-tiled: 4 fp8
# moving cols/cycle, the PE moving-bus limit.  (DoubleRow fp8xfp8 was
# tried and is REJECTED by walrus for tile_position j>0
# (s3d3_mm_valid_dst_partition); untiled DR is only 2 cols/cycle, so it
# cannot beat 4-way column tiling.)
# reduce zone widths per full tile (ACT raw / DVE stt / GPS stt)
# reduce zones per tile: ACT raw-accum [0:ZA), DVE stt pair-add+accum
# [ZA:4096). GPSIMD cannot run TensorScalarPtr (walrus opcode-on-engine
# check), and ACT+DVE alone already outpace the DMA x-rate.
_ZA = int(os.environ.get("KERNEL_ZA", "1352"))
_WARM1 = int(os.environ.get("KERNEL_WARM1", "6"))
_FILL = int(os.environ.get("KERNEL_FILL", "2"))

WSCALE = float(2 ** 20)   # host scale on Abar/qv/u


@functools.lru_cache(maxsize=4)
def _build(za: int = _ZA, warm1: int = _WARM1, nfill: int = _FILL):
    _ensure_path()
    import concourse.bass as bass
    import concourse.tile as tile
    from concourse import bacc, mybir

    f32 = mybir.dt.float32
    bf16 = mybir.dt.bfloat16
    f16 = mybir.dt.float16
    f8 = mybir.dt.float8e4
    ADD = mybir.AluOpType.add
    MUL = mybir.AluOpType.mult
    COPY = mybir.ActivationFunctionType.Copy

    zd = L - za               # DVE stt zone
    zd2 = zd // 2
    # half-tile zones for tile 7 (2048 cols each half)
    ha = za // 2
    hd = H - ha

    nc = bacc.Bacc(
        "TRN2",
        target_bir_lowering=False,
        debug=False,
        enable_asserts=False,
        num_devices=NCORES,
    )

    xT = nc.dram_tensor("xT", [N, L], f8, kind="ExternalInput").ap()
    atr = nc.dram_tensor("atr", [128, PT * N], f8, kind="ExternalInput").ap()
    qv8 = nc.dram_tensor("qv8", [128, PT], f8, kind="ExternalInput").ap()
    u8 = nc.dram_tensor("u8", [128, PT], f32, kind="ExternalInput").ap()
    c0 = nc.dram_tensor("c0", [1, 1], f32, kind="ExternalInput").ap()
    out_d = nc.dram_tensor("out", [1, L], f32, kind="ExternalOutput").ap()

    with tile.TileContext(nc) as tc:
        with (
            tc.tile_pool(name="sb", bufs=1) as sb,
            tc.tile_pool(name="ps", bufs=1, space="PSUM") as ps,
        ):
            xp = [sb.tile([128, 2 * L], f8, tag=f"xp{k}", name=f"xp{k}")
                  for k in range(4)]
            # tile t lives at xp[t//2][:, (t%2)*L : (t%2+1)*L]
            xv = [xp[t // 2][:, (t % 2) * L:(t % 2 + 1) * L] for t in range(8)]
            at0_sb = sb.tile([128, 2 * N], f8, tag="at0")
            at_c = [sb.tile([128, N], f8, tag=f"atc{k}", name=f"atc{k}")
                    for k in range(6)]
            qv_sb = sb.tile([128, PT], f8, tag="qv")
            u_sb = sb.tile([128, PT], f32, tag="u")
            c0_sb = sb.tile([1, 1], f32, tag="c0")
            ones32 = sb.tile([1, 128], f32, tag="ones")
            scrA = sb.tile([128, za], f8, tag="scrA")
            scrD = sb.tile([128, zd2], f16, tag="scrD")
            parts = sb.tile([128, 8, 2], f32, tag="parts")
            parts7 = sb.tile([128, 4], f32, tag="parts7")
            xm7f = sb.tile([128, 1], f32, tag="xm7f")
            xm_all = sb.tile([128, PT], f16, tag="xm")
            w8acc = sb.tile([128, PT], f32, tag="w8acc")
            w_sb = sb.tile([128, PT], bf16, tag="weff", name="w_sb")
            c_sb = sb.tile([1, 1], f32, tag="csb")
            c_bc = sb.tile([128, 1], f32, tag="cbc")
            out_sb = sb.tile([128, 2 * LCH], f32, tag="osb")

            # PSUM: one tile per bank (pad free dim to a full 2KB bank)
            wp_all = ps.tile([128, 512], f32, tag="wp")     # cols 0:64 used
            warm = ps.tile([1, 512], f32, tag="warm")
            c_ps = ps.tile([1, 512], f32, tag="cps")        # [0:1,0:1] used
            cb_ps = ps.tile([128, 512], f32, tag="cbp")     # col 0 used
            o_ps = [ps.tile([128, LCH], f32, tag=f"o{w}", name=f"o{w}")
                    for w in range(2)]

            # ---- DMA: issues pinned to queue fronts ----
            # x rides the two HWDGE rings; at blocks 2-7 dribble in 128KB
            # chunks whose descgens sit in the GPS queue after each
            # combine, so they soak leftover bandwidth mid-stream instead
            # of racing the x tiles, and x7b stays the last arrival.
            with tc.high_priority():
                nc.gpsimd.dma_start(at0_sb[:], atr[:, 0:2 * N])
                nc.gpsimd.dma_start(qv_sb[:], qv8[:])
                nc.gpsimd.dma_start(u_sb[:], u8[:])
                nc.gpsimd.dma_start(c0_sb[:], c0[:])
                for k in range(4):
                    nc.sync.dma_start(
                        xp[k][:, 0:L], xT[256 * k:256 * k + 128, :])
                nc.sync.dma_start(xp[3][:, L + H:2 * L], xT[896:, H:L])
                for k in range(3):
                    nc.scalar.dma_start(
                        xp[k][:, L:2 * L], xT[256 * k + 128:256 * (k + 1), :])
                nc.scalar.dma_start(xp[3][:, L:L + H], xT[896:, 0:H])

            nc.gpsimd.memset(ones32[:], 1.0)
            # hoist ACT table load to t~0 via a dummy activation
            nc.scalar.activation(scrA[0:1, 0:8], ones32[0:1, 0:8], COPY,
                                 bias=0.0)
            # pre-zero pass-2 PSUM rows the matvec leaves unwritten (the
            # epilogue reads all 128 partitions; only rows 32j get data)
            for w in range(2):
                nc.vector.memset(o_ps[w][:, :], 0.0)

            # ---- per-tile row-sum zones (one op per engine) ----
            def reduce_tile(t):
                x_ = xv[t]
                nc.scalar.activation(
                    scrA[:, 0:za], x_[:, 0:za], COPY, bias=0.0,
                    accum_out=parts[:, t, 0:1])
                nc.vector.scalar_tensor_tensor(
                    scrD[:, 0:zd2], x_[:, za:za + zd2], 1.0,
                    x_[:, za + zd2:L], MUL, ADD,
                    accum_out=parts[:, t, 1:2])

            def reduce_half7(h):  # tile 7 halves -> parts7[:, 2h:2h+2]
                x_ = xv[7][:, h * H:(h + 1) * H]
                nc.scalar.activation(
                    scrA[:, 0:ha], x_[:, 0:ha], COPY, bias=0.0,
                    accum_out=parts7[:, 2 * h + 0:2 * h + 1])
                hd2 = hd // 2
                nc.vector.scalar_tensor_tensor(
                    scrD[:, 0:hd2], x_[:, ha:ha + hd2], 1.0,
                    x_[:, ha + hd2:H], MUL, ADD,
                    accum_out=parts7[:, 2 * h + 1:2 * h + 2])

            def combine(t):  # xm[t] = A + D on GPS, fp16
                nc.gpsimd.tensor_add(
                    xm_all[:, t:t + 1], parts[:, t, 0:1], parts[:, t, 1:2])

            # ---- folds ----
            def fold(pt):
                if pt < 2:
                    a_sb, off = at0_sb, pt
                else:
                    a_sb, off = at_c[pt - 2], 0
                for nt in range(PT):
                    nc.tensor.matmul(
                        wp_all[:, pt * 8 + nt:pt * 8 + nt + 1],
                        a_sb[:, off * N + nt * 128:off * N + (nt + 1) * 128],
                        xm_all[:, pt:pt + 1], start=True, stop=True)
                nc.tensor.matmul(
                    c_ps[0:1, 0:1], qv_sb[:, pt:pt + 1], xm_all[:, pt:pt + 1],
                    start=(pt == 0), stop=(pt == PT - 1))
                # PE fillers gated on this xm: keep the HAM clock ramping
                if pt < 6:
                    for i in range(nfill):
                        nc.tensor.matmul(
                            warm[0:1, :], xm_all[:, pt:pt + 1],
                            xv[pt][:, i * LCH:(i + 1) * LCH],
                            start=(i == 0), stop=(i == nfill - 1))

            # warmup burst as soon as x0 lands
            for i in range(warm1):
                nc.tensor.matmul(
                    warm[0:1, :], xv[0][:, 0:1], xv[0][:, 0:LCH],
                    start=(i == 0), stop=(i == warm1 - 1))

            # tiles 0..6 (pair-buffered), tile 7 in halves
            for t in range(7):
                reduce_tile(t)
                if t >= 1:
                    combine(t - 1)
                    if t - 1 < 6:
                        # gps-queue-gated descgen: at chunk t+1 starts
                        # moving only once combine(t-1) has retired
                        nc.gpsimd.dma_start(
                            at_c[t - 1][:], atr[:, (t + 1) * N:(t + 2) * N])
                    fold(t - 1)
            reduce_half7(0)
            # late warm burst: x6 is resident; keep the PE pstate streak
            # alive across the fold-6/fold-7 window into pass-2
            for i in range(6):
                nc.tensor.matmul(
                    warm[0:1, :], xv[6][:, 1:2], xv[6][:, i * LCH:(i + 1) * LCH],
                    start=(i == 0), stop=(i == 5))
            combine(6)
            fold(6)
            reduce_half7(1)
            for i in range(3):
                nc.tensor.matmul(
                    warm[0:1, :], xv[7][:, 1:2], xv[7][:, i * LCH:(i + 1) * LCH],
                    start=(i == 0), stop=(i == 2))
            # pre-sum folds 0-6 plus u while tile 7 is still reducing: the
            # post-fold-7 critical chain is then a single stt off PSUM
            nc.vector.tensor_reduce(
                w8acc[:], wp_all[:, 0:56].rearrange("p (a b) -> p b a", a=7),
                axis=mybir.AxisListType.X, op=mybir.AluOpType.add)
            nc.vector.tensor_add(w8acc[:], w8acc[:], u_sb[:])
            # tile-7 combine on DVE (GPS may lag; DVE is free now)
            with nc.allow_low_precision("xm7 fp16 accumulate, 4 partials"):
                nc.vector.tensor_reduce(
                    xm_all[:, 7:8], parts7[:], axis=mybir.AxisListType.X,
                    op=mybir.AluOpType.add)
            fold(7)

            # ---- finalize w / c ----
            nc.vector.scalar_tensor_tensor(
                w_sb[:], wp_all[:, 56:64], 1.0, w8acc[:], MUL, ADD)
            nc.vector.tensor_scalar(
                c_sb[:], c_ps[0:1, 0:1], 1.0 / WSCALE, c0_sb[0:1, 0:1],
                MUL, ADD)
            # broadcast c to all partitions via PE + ACT copy
            nc.tensor.matmul(cb_ps[:, 0:1], ones32[:], c_sb[:],
                             start=True, stop=True)
            nc.scalar.activation(c_bc[:], cb_ps[:, 0:1], COPY, bias=0.0)

            # ---- pass 2 ----
            oscale = 1.0 / WSCALE
            for wave in range(2):
                for nt in range(PT):
                    for j in range(4):
                        lc = wave * 4 + j
                        nc.tensor.matmul(
                            o_ps[wave][32 * j:32 * j + 1, :],
                            w_sb[:, nt:nt + 1],
                            xv[nt][:, lc * LCH:(lc + 1) * LCH],
                            start=(nt == 0), stop=(nt == PT - 1),
                            tile_position=(0, 32 * j))
                nc.vector.tensor_scalar(
                    out_sb[:, wave * LCH:(wave + 1) * LCH], o_ps[wave][:, :],
                    oscale, c_bc[:, 0:1], MUL, ADD)
                nc.sync.dma_start(
                    out_d[0:1, wave * 4 * LCH:(wave + 1) * 4 * LCH]
                    .rearrange("p (j k) -> p j k", j=4),
                    out_sb[0:97:32, wave * LCH:(wave + 1) * LCH])

    nc.compile()
    return nc


def _prep_host(inputs):
    """Fold weights on host (f64 accumulate) and lay out per-core arrays."""
    import ml_dtypes

    Wq = np.asarray(inputs["Wq"], np.float64)
    bq = np.asarray(inputs["bq"], np.float64)
    Wk = np.asarray(inputs["Wk"], np.float64)
    bk = np.asarray(inputs["bk"], np.float64)
    Wfc = np.asarray(inputs["Wfc"], np.float64)
    bfc = np.asarray(inputs["bfc"], np.float64)

    s = np.repeat(Wfc[0], D_K) / np.sqrt(D_K)
    A = (Wk * s[:, None]).T @ Wq / L          # [n, p] ; w_eff = A @ xsum + u
    u = Wk.T @ (s * bq)
    qv = Wq.T @ (s * bk) / L
    c0 = float((s * bk) @ bq + bfc[0])

    f8 = ml_dtypes.float8_e4m3

    at = np.ascontiguousarray(A.T) * WSCALE
    atr = np.ascontiguousarray(
        at.reshape(PT, 128, N).transpose(1, 0, 2).reshape(128, PT * N)
    ).astype(f8)
    qv8 = np.ascontiguousarray(
        (qv * WSCALE).reshape(PT, 128).T).astype(f8)
    u8 = np.ascontiguousarray(
        (u * WSCALE).reshape(PT, 128).T).astype(np.float32)
    c0a = np.full((1, 1), c0, np.float32)

    x = np.asarray(inputs["x"])
    shared = {"atr": atr, "qv8": qv8, "u8": u8, "c0": c0a}
    in_maps = []
    for c in range(NCORES):
        m = dict(shared)
        m["xT"] = np.ascontiguousarray(x[c].T).astype(f8)
        in_maps.append(m)
    return in_maps


LAST_RESULTS = None


def kernel(**inputs) -> np.ndarray:
    global LAST_RESULTS
    _ensure_path()
    from concourse.bass_utils import run_bass_kernel_spmd

    nc = _build()
    in_maps = _prep_host(inputs)
    kw = {}
    if os.environ.get("KERNEL_TRACE"):
        kw["trace"] = True
    res = run_bass_kernel_spmd(nc, in_maps, list(range(NCORES)), **kw)
    LAST_RESULTS = res
    out = np.stack([res.results[c]["out"].reshape(L, 1) for c in range(NCORES)])
    return out.astype(np.float32)


if __name__ == "__main__":
    rng = np.random.default_rng(0)
    demo = {
        "x": rng.standard_normal((B, L, N), np.float32),
        "Wq": rng.standard_normal((N, N), np.float32) * 0.03,
        "bq": rng.standard_normal((N,), np.float32) * 0.03,
        "Wk": rng.standard_normal((N, N), np.float32) * 0.03,
        "bk": rng.standard_normal((N,), np.float32) * 0.03,
        "Wfc": rng.standard_normal((1, 16), np.float32) * 0.25,
        "bfc": rng.standard_normal((1,), np.float32) * 0.25,
    }
    o = kernel(**demo)
    print("out", o.shape, o.dtype, float(np.abs(o).max()))


# revision 17
# speedup vs baseline: 1.1501x; 1.0118x over previous
"""Trainium2 Bass kernel for nn_MultiHeadAttention_8684423872640.

Math: the reference collapses algebraically. With
  s[m]   = Wfc[0, m // 64] / sqrt(64)
  Abar   = (Wk * s[:,None]).T @ Wq / L          # [1024, 1024] weights-only
  u      = Wk.T @ (s * bq)                      # [1024]
  qv     = Wq.T @ (s * bk) / L                  # [1024]
  c0     = (s * bk) @ bq + bfc[0]
the output for batch b is
  xsum_b = sum_l x[b, l, :]                     # [1024]
  w_eff  = Abar @ xsum_b + u                    # [1024]
  c      = qv @ xsum_b + c0
  out[b, l, 0] = x[b, l, :] @ w_eff + c

Sharding: data-parallel over B -- core c handles batch c.

v2 pipeline (per core):
  - x ships fp8-e4m3 [N=1024, L=4096] as 4 pair tiles [128, 2L]; rings:
    sync x0,x2,x4,x6,x7b / scalar x1,x3,x5,x7a / gps at0,qv,u,c0,at1.
    All DMA issues pinned to queue fronts (descgen done before data
    arrives, so compute engines' queues are clean in steady state).
  - Row sums: per tile, three zones reduced in parallel, one op each:
    ACT activation(Copy, accum_out) on raw fp8; DVE and GPS each do
    scalar_tensor_tensor(halves, op=add, accum_out) -- pairwise add +
    free-axis accumulate in a single pass (2 cols/cycle consumed).
    GPS combines the three fp32 partials into fp16 xm in one stt op.
  - Folds: per tile pt, 8 closed-group MMs (at fp8 x xm fp16) into a
    single PSUM bank (cols pt*8+nt) + qv MM chained into cps.  One DVE
    tensor_reduce at the end sums all 64 fold columns -> w8acc, then
    one stt adds u and rescales into the pass-2 w dtype.
  - PE warmup MMs gated on x0/x2 keep the HAM clock ramping through
    the reduce window (fillers per fold as in v1).
  - c broadcast via PE ones-matmul + ACT copy.
  - Pass-2 (KERNEL_P2=dr, default): DoubleRow fp8xfp8 matvec -- 32 MMs
    of [128,2,512] pairs at tile_position (0,32j), 2 PSUM waves; w in
    fp8 scaled 2^18.  KERNEL_P2=mx falls back to v1's 64 mixed-dtype
    MMs (bf16 w, scale 2^20).
  - Epilogue per wave (tensor_scalar mult+add c) overlaps wave 2 MMs;
    two simple 4-descriptor out-DMAs on the sync ring.
"""

import os
import sys
import functools
import numpy as np

B, L, N = 8, 4096, 1024
D_K = 64
NCORES = 8
PT = N // 128   # 8 feature tiles
LCH = 512       # pass-2 moving chunk (PSUM bank limit)
H = L // 2

_TRN_REPO = "/opt/trn_rl_repo"


def _ensure_path():
    if _TRN_REPO not in sys.path and os.path.isdir(_TRN_REPO):
        sys.path.insert(0, _TRN_REPO)


# pass-2 is bf16-w-stationary x fp8-moving, 4-way column-tiled: 4 fp8
# moving cols/cycle, the PE moving-bus limit.  (DoubleRow fp8xfp8 was
# tried and is REJECTED by walrus for tile_position j>0
# (s3d3_mm_valid_dst_partition); untiled DR is only 2 cols/cycle, so it
# cannot beat 4-way column tiling.)
# reduce zone widths per full tile (ACT raw / DVE stt / GPS stt)
# reduce zones per tile: ACT raw-accum [0:ZA), DVE stt pair-add+accum
# [ZA:4096). GPSIMD cannot run TensorScalarPtr (walrus opcode-on-engine
# check), and ACT+DVE alone already outpace the DMA x-rate.
_ZA = int(os.environ.get("KERNEL_ZA", "1352"))
_WARM1 = int(os.environ.get("KERNEL_WARM1", "6"))
_FILL = int(os.environ.get("KERNEL_FILL", "2"))

WSCALE = float(2 ** 20)   # host scale on Abar/qv/u


@functools.lru_cache(maxsize=4)
def _build(za: int = _ZA, warm1: int = _WARM1, nfill: int = _FILL):
    _ensure_path()
    import concourse.bass as bass
    import concourse.tile as tile
    from concourse import bacc, mybir

    f32 = mybir.dt.float32
    bf16 = mybir.dt.bfloat16
    f16 = mybir.dt.float16
    f8 = mybir.dt.float8e4
    ADD = mybir.AluOpType.add
    MUL = mybir.AluOpType.mult
    COPY = mybir.ActivationFunctionType.Copy

    zd = L - za               # DVE stt zone
    zd2 = zd // 2
    # half-tile zones for tile 7 (2048 cols each half)
    ha = za // 2
    hd = H - ha

    nc = bacc.Bacc(
        "TRN2",
        target_bir_lowering=False,
        debug=False,
        enable_asserts=False,
        num_devices=NCORES,
    )

    xT = nc.dram_tensor("xT", [N, L], f8, kind="ExternalInput").ap()
    atr = nc.dram_tensor("atr", [128, PT * N], f8, kind="ExternalInput").ap()
    qv8 = nc.dram_tensor("qv8", [128, PT], f8, kind="ExternalInput").ap()
    u8 = nc.dram_tensor("u8", [128, PT], f32, kind="ExternalInput").ap()
    c0 = nc.dram_tensor("c0", [1, 1], f32, kind="ExternalInput").ap()
    out_d = nc.dram_tensor("out", [1, L], f32, kind="ExternalOutput").ap()

    with tile.TileContext(nc) as tc:
        with (
            tc.tile_pool(name="sb", bufs=1) as sb,
            tc.tile_pool(name="ps", bufs=1, space="PSUM") as ps,
        ):
            xp = [sb.tile([128, 2 * L], f8, tag=f"xp{k}", name=f"xp{k}")
                  for k in range(4)]
            # tile t lives at xp[t//2][:, (t%2)*L : (t%2+1)*L]
            xv = [xp[t // 2][:, (t % 2) * L:(t % 2 + 1) * L] for t in range(8)]
            at0_sb = sb.tile([128, 2 * N], f8, tag="at0")
            at_c = [sb.tile([128, N], f8, tag=f"atc{k}", name=f"atc{k}")
                    for k in range(6)]
            qv_sb = sb.tile([128, PT], f8, tag="qv")
            u_sb = sb.tile([128, PT], f32, tag="u")
            c0_sb = sb.tile([1, 1], f32, tag="c0")
            ones32 = sb.tile([1, 128], f32, tag="ones")
            scrA = sb.tile([128, za], f8, tag="scrA")
            scrD = sb.tile([128, zd2], f16, tag="scrD")
            parts = sb.tile([128, 8, 2], f32, tag="parts")
            parts7 = sb.tile([128, 4], f32, tag="parts7")
            xm7f = sb.tile([128, 1], f32, tag="xm7f")
            xm_all = sb.tile([128, PT], f16, tag="xm")
            w8acc = sb.tile([128, PT], f32, tag="w8acc")
            w_sb = sb.tile([128, PT], bf16, tag="weff", name="w_sb")
            c_sb = sb.tile([1, 1], f32, tag="csb")
            c_bc = sb.tile([128, 1], f32, tag="cbc")
            out_sb = sb.tile([128, 2 * LCH], f32, tag="osb")

            # PSUM: one tile per bank (pad free dim to a full 2KB bank)
            wp_all = ps.tile([128, 512], f32, tag="wp")     # cols 0:64 used
            warm = ps.tile([1, 512], f32, tag="warm")
            c_ps = ps.tile([1, 512], f32, tag="cps")        # [0:1,0:1] used
            cb_ps = ps.tile([128, 512], f32, tag="cbp")     # col 0 used
            o_ps = [ps.tile([128, LCH], f32, tag=f"o{w}", name=f"o{w}")
                    for w in range(2)]

            # ---- DMA: issues pinned to queue fronts ----
            # x rides the two HWDGE rings; at blocks 2-7 dribble in 128KB
            # chunks whose descgens sit in the GPS queue after each
            # combine, so they soak leftover bandwidth mid-stream instead
            # of racing the x tiles, and x7b stays the last arrival.
            with tc.high_priority():
                nc.gpsimd.dma_start(at0_sb[:], atr[:, 0:2 * N])
                nc.gpsimd.dma_start(qv_sb[:], qv8[:])
                nc.gpsimd.dma_start(u_sb[:], u8[:])
                nc.gpsimd.dma_start(c0_sb[:], c0[:])
                for k in range(4):
                    nc.sync.dma_start(
                        xp[k][:, 0:L], xT[256 * k:256 * k + 128, :])
                nc.sync.dma_start(xp[3][:, L + H:2 * L], xT[896:, H:L])
                for k in range(3):
                    nc.scalar.dma_start(
                        xp[k][:, L:2 * L], xT[256 * k + 128:256 * (k + 1), :])
                nc.scalar.dma_start(xp[3][:, L:L + H], xT[896:, 0:H])

            nc.gpsimd.memset(ones32[:], 1.0)
            # hoist ACT table load to t~0 via a dummy activation
            nc.scalar.activation(scrA[0:1, 0:8], ones32[0:1, 0:8], COPY,
                                 bias=0.0)
            # pre-zero pass-2 PSUM rows the matvec leaves unwritten (the
            # epilogue reads all 128 partitions; only rows 32j get data)
            for w in range(2):
                nc.vector.memset(o_ps[w][:, :], 0.0)

            # ---- per-tile row-sum zones (one op per engine) ----
            def reduce_tile(t):
                x_ = xv[t]
                nc.scalar.activation(
                    scrA[:, 0:za], x_[:, 0:za], COPY, bias=0.0,
                    accum_out=parts[:, t, 0:1])
                nc.vector.scalar_tensor_tensor(
                    scrD[:, 0:zd2], x_[:, za:za + zd2], 1.0,
                    x_[:, za + zd2:L], MUL, ADD,
                    accum_out=parts[:, t, 1:2])

            def reduce_half7(h):  # tile 7 halves -> parts7[:, 2h:2h+2]
                x_ = xv[7][:, h * H:(h + 1) * H]
                nc.scalar.activation(
                    scrA[:, 0:ha], x_[:, 0:ha], COPY, bias=0.0,
                    accum_out=parts7[:, 2 * h + 0:2 * h + 1])
                hd2 = hd // 2
                nc.vector.scalar_tensor_tensor(
                    scrD[:, 0:hd2], x_[:, ha:ha + hd2], 1.0,
                    x_[:, ha + hd2:H], MUL, ADD,
                    accum_out=parts7[:, 2 * h + 1:2 * h + 2])

            def combine(t):  # xm[t] = A + D on GPS, fp16
                nc.gpsimd.tensor_add(
                    xm_all[:, t:t + 1], parts[:, t, 0:1], parts[:, t, 1:2])

            # ---- folds ----
            def fold(pt):
                if pt < 2:
                    a_sb, off = at0_sb, pt
                else:
                    a_sb, off = at_c[pt - 2], 0
                for nt in range(PT):
                    nc.tensor.matmul(
                        wp_all[:, pt * 8 + nt:pt * 8 + nt + 1],
                        a_sb[:, off * N + nt * 128:off * N + (nt + 1) * 128],
                        xm_all[:, pt:pt + 1], start=True, stop=True)
                nc.tensor.matmul(
                    c_ps[0:1, 0:1], qv_sb[:, pt:pt + 1], xm_all[:, pt:pt + 1],
                    start=(pt == 0), stop=(pt == PT - 1))
                # PE fillers gated on this xm: keep the HAM clock ramping
                if pt < 6:
                    for i in range(nfill):
                        nc.tensor.matmul(
                            warm[0:1, :], xm_all[:, pt:pt + 1],
                            xv[pt][:, i * LCH:(i + 1) * LCH],
                            start=(i == 0), stop=(i == nfill - 1))

            # warmup burst as soon as x0 lands
            for i in range(warm1):
                nc.tensor.matmul(
                    warm[0:1, :], xv[0][:, 0:1], xv[0][:, 0:LCH],
                    start=(i == 0), stop=(i == warm1 - 1))

            # tiles 0..6 (pair-buffered), tile 7 in halves
            for t in range(7):
                reduce_tile(t)
                if t >= 1:
                    combine(t - 1)
                    if t - 1 < 6:
                        # gps-queue-gated descgen: at chunk t+1 starts
                        # moving only once combine(t-1) has retired
                        nc.gpsimd.dma_start(
                            at_c[t - 1][:], atr[:, (t + 1) * N:(t + 2) * N])
                    fold(t - 1)
            reduce_half7(0)
            # late warm burst: x6 is resident; keep the PE pstate streak
            # alive across the fold-6/fold-7 window into pass-2
            for i in range(4):
                nc.tensor.matmul(
                    warm[0:1, :], xv[6][:, 1:2], xv[6][:, i * LCH:(i + 1) * LCH],
                    start=(i == 0), stop=(i == 3))
            combine(6)
            fold(6)
            reduce_half7(1)
            for i in range(2):
                nc.tensor.matmul(
                    warm[0:1, :], xv[7][:, 1:2], xv[7][:, i * LCH:(i + 1) * LCH],
                    start=(i == 0), stop=(i == 1))
            # tile-7 combine FIRST on DVE: fold(7) must not sit behind the
            # w8acc pre-reduce (which waits on fold-6 retiring on the PE)
            with nc.allow_low_precision("xm7 fp16 accumulate, 4 partials"):
                nc.vector.tensor_reduce(
                    xm_all[:, 7:8], parts7[:], axis=mybir.AxisListType.X,
                    op=mybir.AluOpType.add)
            fold(7)
            # pre-sum folds 0-6 plus u while fold-7 streams: the post-
            # fold-7 critical chain is then a single stt off PSUM
            nc.vector.tensor_reduce(
                w8acc[:], wp_all[:, 0:56].rearrange("p (a b) -> p b a", a=7),
                axis=mybir.AxisListType.X, op=mybir.AluOpType.add)
            nc.vector.tensor_add(w8acc[:], w8acc[:], u_sb[:])

            # ---- finalize w / c ----
            nc.vector.scalar_tensor_tensor(
                w_sb[:], wp_all[:, 56:64], 1.0, w8acc[:], MUL, ADD)
            nc.vector.tensor_scalar(
                c_sb[:], c_ps[0:1, 0:1], 1.0 / WSCALE, c0_sb[0:1, 0:1],
                MUL, ADD)

            # ---- pass 2 ----
            # c-broadcast (PE ones-MM + ACT copy) is emitted BETWEEN the
            # waves: it must not gate wave-0 MMs on the c path, and the
            # wave-0 epilogue that consumes c_bc runs much later anyway.
            oscale = 1.0 / WSCALE
            for wave in range(2):
                for nt in range(PT):
                    for j in range(4):
                        lc = wave * 4 + j
                        nc.tensor.matmul(
                            o_ps[wave][32 * j:32 * j + 1, :],
                            w_sb[:, nt:nt + 1],
                            xv[nt][:, lc * LCH:(lc + 1) * LCH],
                            start=(nt == 0), stop=(nt == PT - 1),
                            tile_position=(0, 32 * j))
                if wave == 0:
                    nc.tensor.matmul(cb_ps[:, 0:1], ones32[:], c_sb[:],
                                     start=True, stop=True)
                    nc.scalar.activation(c_bc[:], cb_ps[:, 0:1], COPY,
                                         bias=0.0)
                nc.vector.tensor_scalar(
                    out_sb[:, wave * LCH:(wave + 1) * LCH], o_ps[wave][:, :],
                    oscale, c_bc[:, 0:1], MUL, ADD)
                nc.sync.dma_start(
                    out_d[0:1, wave * 4 * LCH:(wave + 1) * 4 * LCH]
                    .rearrange("p (j k) -> p j k", j=4),
                    out_sb[0:97:32, wave * LCH:(wave + 1) * LCH])

    nc.compile()
    return nc


def _prep_host(inputs):
    """Fold weights on host (f64 accumulate) and lay out per-core arrays."""
    import ml_dtypes

    Wq = np.asarray(inputs["Wq"], np.float64)
    bq = np.asarray(inputs["bq"], np.float64)
    Wk = np.asarray(inputs["Wk"], np.float64)
    bk = np.asarray(inputs["bk"], np.float64)
    Wfc = np.asarray(inputs["Wfc"], np.float64)
    bfc = np.asarray(inputs["bfc"], np.float64)

    s = np.repeat(Wfc[0], D_K) / np.sqrt(D_K)
    A = (Wk * s[:, None]).T @ Wq / L          # [n, p] ; w_eff = A @ xsum + u
    u = Wk.T @ (s * bq)
    qv = Wq.T @ (s * bk) / L
    c0 = float((s * bk) @ bq + bfc[0])

    f8 = ml_dtypes.float8_e4m3

    at = np.ascontiguousarray(A.T) * WSCALE
    atr = np.ascontiguousarray(
        at.reshape(PT, 128, N).transpose(1, 0, 2).reshape(128, PT * N)
    ).astype(f8)
    qv8 = np.ascontiguousarray(
        (qv * WSCALE).reshape(PT, 128).T).astype(f8)
    u8 = np.ascontiguousarray(
        (u * WSCALE).reshape(PT, 128).T).astype(np.float32)
    c0a = np.full((1, 1), c0, np.float32)

    x = np.asarray(inputs["x"])
    shared = {"atr": atr, "qv8": qv8, "u8": u8, "c0": c0a}
    in_maps = []
    for c in range(NCORES):
        m = dict(shared)
        m["xT"] = np.ascontiguousarray(x[c].T).astype(f8)
        in_maps.append(m)
    return in_maps


LAST_RESULTS = None


def kernel(**inputs) -> np.ndarray:
    global LAST_RESULTS
    _ensure_path()
    from concourse.bass_utils import run_bass_kernel_spmd

    nc = _build()
    in_maps = _prep_host(inputs)
    kw = {}
    if os.environ.get("KERNEL_TRACE"):
        kw["trace"] = True
    res = run_bass_kernel_spmd(nc, in_maps, list(range(NCORES)), **kw)
    LAST_RESULTS = res
    out = np.stack([res.results[c]["out"].reshape(L, 1) for c in range(NCORES)])
    return out.astype(np.float32)


if __name__ == "__main__":
    rng = np.random.default_rng(0)
    demo = {
        "x": rng.standard_normal((B, L, N), np.float32),
        "Wq": rng.standard_normal((N, N), np.float32) * 0.03,
        "bq": rng.standard_normal((N,), np.float32) * 0.03,
        "Wk": rng.standard_normal((N, N), np.float32) * 0.03,
        "bk": rng.standard_normal((N,), np.float32) * 0.03,
        "Wfc": rng.standard_normal((1, 16), np.float32) * 0.25,
        "bfc": rng.standard_normal((1,), np.float32) * 0.25,
    }
    o = kernel(**demo)
    print("out", o.shape, o.dtype, float(np.abs(o).max()))
